# revision 2
# baseline (speedup 1.0000x reference)
"""Trainium2 Bass kernel for BiDAF-style bidirectional attention.

Reference math (per batch b):
    sim[c,q]  = q[q]·wq + c[c]·wc + sum_e wm[e]*question[q,e]*context[c,e]
    c2q[c,:]  = softmax_q(sim[c,:]) @ question          # (C, E)
    q2c[:]    = softmax_c(max_q sim[c,:]) @ context     # (E,)
    out[c,:]  = [context | c2q | context*c2q | context*q2c]

Sharding: pure data parallel over batch (B=16 -> 2 batches per core x 8 cores).

Design notes (v2 — tuned against the ntff profile of the first version):
  - ctx/out DRAM and the staging tiles are float32r: the PE consumes the
    DMA-written bits directly (sim stationary via transpose, q2c chain
    moving operand), eliminating the bf16 cast sweep the old version ran
    on the vector engine (22us).
  - the attention-weight path (P, its transpose, question operand of the
    c2q matmul) runs in fp16: 1 cyc/row on the PE and half the copy bytes,
    with ~2^-11 rounding — well inside the 2e-2 gate.
  - per-batch epilogue: batch 0's q2c reduction, ctx*q2c products and
    output writes overlap batch 1's main pass instead of trailing the
    whole kernel.
  - input DMAs are dispatched first (sync queue) or prefetched mid-pass
    from the scalar engine's DGE queue so input transfers overlap the
    output stream on the sync queue.
"""

import numpy as np

import concourse.bass as bass
import concourse.tile as tile
import concourse.mybir as mybir
from concourse import bacc
from concourse.bass_utils import run_bass_kernel_spmd
from concourse.masks import make_identity

B, C, Q, E = 16, 2048, 128, 256
NCORES = 8
BPC = B // NCORES          # batches per core
NT = C // 128              # context tiles per batch
NG = NT // 4               # groups of 4 tiles
F32 = mybir.dt.float32
F32R = mybir.dt.float32r
F16 = mybir.dt.float16


def _body(tc, out_ext, ctx_in, q_in, wq_in, wc_in, wm_in):
    nc = tc.nc
    with (
        tc.tile_pool(name="singles", bufs=1) as singles,
        tc.tile_pool(name="xcp", bufs=1) as xcp,
        tc.tile_pool(name="qside", bufs=2) as qside,
        tc.tile_pool(name="work", bufs=4) as work,
        tc.tile_pool(name="statsp", bufs=1) as statsp,
        tc.tile_pool(name="ps_xct", bufs=2, space="PSUM") as ps_xct,
        tc.tile_pool(name="ps_sim", bufs=2, space="PSUM") as ps_sim,
        tc.tile_pool(name="ps_pt", bufs=1, space="PSUM") as ps_pt,
        tc.tile_pool(name="ps_c2q", bufs=1, space="PSUM") as ps_c2q,
        tc.tile_pool(name="ps_q2c", bufs=1, space="PSUM") as ps_q2c,
        tc.tile_pool(name="ps_misc", bufs=1, space="PSUM") as ps_misc,
    ):
        # ---- staging tiles + input DMA dispatch (before anything else) ----
        stgs = {}
        for b in range(BPC):
            for g in range(NG):
                stgs[b, g] = xcp.tile(
                    [128, 4, 4 * E], F32R, tag="stg", bufs=2 * NG,
                    name=f"stg{b}{g}",
                )
        qms = {}
        for b in range(BPC):
            qms[b] = qside.tile([128, E], F32, tag="qm", name=f"qm{b}")
        # early loads on the sync DGE queue (idle until outputs start)
        for b in range(BPC):
            nc.sync.dma_start(out=qms[b], in_=q_in[b])
        for g in range(NG):
            nc.sync.dma_start(
                out=stgs[0, g][:, :, 0:E],
                in_=ctx_in[0, g * 512 : (g + 1) * 512, :].rearrange(
                    "(t p) e -> p t e", p=128
                ),
            )
        wq_sb = singles.tile([128, 2], F32)
        nc.sync.dma_start(out=wq_sb, in_=wq_in.rearrange("(j p) -> p j", p=128))
        wc_sb = singles.tile([128, 2], F32)
        nc.sync.dma_start(out=wc_sb, in_=wc_in.rearrange("(j p) -> p j", p=128))
        wm_sb = singles.tile([128, 2], F32)
        nc.sync.dma_start(out=wm_sb, in_=wm_in.rearrange("(j p) -> p j", p=128))

        # batch-1 context loads are prefetched mid-pass from the scalar DGE
        # queue so they overlap the batch-0 output stream (issued below).

        ident = singles.tile([128, 128], F32)
        make_identity(nc, ident)
        ident_h = singles.tile([128, 128], F16)
        make_identity(nc, ident_h)
        ones_r = singles.tile([1, 128], F32)
        nc.vector.memset(ones_r, 1.0)
        ones_c = singles.tile([128, 1], F32)
        nc.vector.memset(ones_c, 1.0)

        def phase_a(b):
            """Question-side prep for batch b."""
            qm = qms[b]
            qm_h = qside.tile([128, E], F16, tag="qm_h", name=f"qmh{b}")
            nc.vector.tensor_copy(out=qm_h, in_=qm)
            qmt_ps = ps_misc.tile([128, E], F32, tag="misc", name=f"qmtp{b}")
            for j in range(2):
                nc.tensor.transpose(
                    qmt_ps[:, j * 128 : (j + 1) * 128],
                    qm[:, j * 128 : (j + 1) * 128],
                    ident,
                )
            qmt_sb = qside.tile([128, E], F32, tag="qmt", name=f"qmt{b}")
            nc.vector.tensor_copy(out=qmt_sb, in_=qmt_ps)
            rhs_aug = qside.tile([128, 2, E], F32R, tag="rhs", name=f"rhs{b}")
            for j in range(2):
                nc.vector.tensor_scalar_mul(
                    rhs_aug[:, j, 0:128],
                    qmt_sb[:, j * 128 : (j + 1) * 128],
                    wm_sb[:, j : j + 1],
                )
                nc.vector.tensor_copy(
                    out=rhs_aug[:, j, 128:129], in_=wc_sb[:, j : j + 1]
                )
                nc.vector.tensor_scalar_mul(
                    rhs_aug[:, j, 129:256],
                    qmt_sb[:, j * 128 : (j + 1) * 128][:, 0:127],
                    0.0,
                )
            qw_ps = ps_misc.tile([1, 128], F32, tag="misc", name=f"qwp{b}")
            for j in range(2):
                nc.tensor.matmul(
                    qw_ps,
                    wq_sb[:, j : j + 1],
                    qmt_sb[:, j * 128 : (j + 1) * 128],
                    start=(j == 0),
                    stop=(j == 1),
                )
            qw_row = qside.tile([1, 128], F32, tag="qw_row", name=f"qwr{b}")
            nc.vector.tensor_copy(out=qw_row, in_=qw_ps)
            qwb_ps = ps_misc.tile([128, 128], F32, tag="misc", name=f"qwbp{b}")
            nc.tensor.matmul(qwb_ps, ones_r, qw_row, start=True, stop=True)
            qw_bcast2 = qside.tile([128, 2, 128], F32, tag="qwb", name=f"qwb{b}")
            nc.vector.tensor_copy(out=qw_bcast2[:, 0, :], in_=qwb_ps)
            nc.vector.tensor_copy(out=qw_bcast2[:, 1, :], in_=qwb_ps)
            return qm_h, rhs_aug, qw_bcast2

        def pass1(b, qm_h, rhs_aug, qw_bcast2, mstat, row_sum, recip):
            for g in range(NG):
                stg = stgs[b, g]
                for h in range(2):
                    sim_ps = ps_sim.tile([128, 2, E], F32, tag="sim")
                    for i in range(2):
                        lane = 2 * h + i
                        xc = stg[:, lane, 0:E].bitcast(F32)
                        xct_ps = ps_xct.tile([128, E], F32, tag="xct")
                        for j in range(2):
                            nc.tensor.transpose(
                                xct_ps[:, j * 128 : (j + 1) * 128],
                                xc[:, j * 128 : (j + 1) * 128],
                                ident,
                            )
                        xct_sb = work.tile([128, E], F32R, tag="xct_sb")
                        nc.vector.tensor_copy(out=xct_sb, in_=xct_ps)
                        for j in range(2):
                            nc.tensor.matmul(
                                sim_ps[:, i, :],
                                xct_sb[:, j * 128 : (j + 1) * 128],
                                rhs_aug[:, j, :],
                                start=(j == 0),
                                stop=(j == 1),
                            )
                    t0 = 4 * g + 2 * h
                    sim_in = work.tile([128, 2, 128], F32, tag="sim_in")
                    nc.vector.tensor_add(sim_in, sim_ps[:, :, 0:128], qw_bcast2)
                    neg_m = work.tile([128, 2], F32, tag="neg_m")
                    nc.vector.reduce_max(
                        out=neg_m,
                        in_=sim_in,
                        axis=mybir.AxisListType.X,
                        negate=True,
                    )
                    # mstat = cw + max(sim+qw): full row-max for the q2c softmax
                    nc.vector.tensor_sub(
                        mstat[:, t0 : t0 + 2], sim_ps[:, :, 128], neg_m
                    )
                    p_sb = work.tile([128, 2, 128], F16, tag="p_sb")
                    for i in range(2):
                        nc.scalar.activation(
                            out=p_sb[:, i, :],
                            in_=sim_in[:, i, :],
                            func=mybir.ActivationFunctionType.Exp,
                            bias=neg_m[:, i : i + 1],
                            scale=1.0,
                            accum_out=row_sum[:, t0 + i : t0 + i + 1],
                        )
                    nc.vector.reciprocal(
                        out=recip[:, t0 : t0 + 2], in_=row_sum[:, t0 : t0 + 2]
                    )
                    for i in range(2):
                        lane = 2 * h + i
                        pt_ps = ps_pt.tile([128, 128], F16, tag="pt")
                        nc.tensor.transpose(pt_ps, p_sb[:, i, :], ident_h)
                        pt_sb = work.tile([128, 128], F16, tag="pt_sb")
                        nc.vector.tensor_copy(out=pt_sb, in_=pt_ps)
                        c2q_ps = ps_c2q.tile([128, E], F32, tag="c2q")
                        nc.tensor.matmul(c2q_ps, pt_sb, qm_h, start=True, stop=True)
                        nc.scalar.activation(
                            out=stg[:, lane, E : 2 * E],
                            in_=c2q_ps,
                            func=mybir.ActivationFunctionType.Copy,
                            scale=recip[:, t0 + i : t0 + i + 1],
                        )
                # ctx * c2q for the group; wide store of cols 0:768
                nc.gpsimd.tensor_mul(
                    stg[:, :, 2 * E : 3 * E],
                    stg[:, :, 0:E],
                    stg[:, :, E : 2 * E],
                )
                nc.sync.dma_start(
                    out=out_ext[b, g * 512 : (g + 1) * 512, 0 : 3 * E].rearrange(
                        "(t p) f -> p t f", p=128
                    ),
                    in_=stg[:, :, 0 : 3 * E],
                )
                # prefetch batch-1 context on the scalar DGE queue
                if b == 0:
                    nc.scalar.dma_start(
                        out=stgs[1, g][:, :, 0:E],
                        in_=ctx_in[1, g * 512 : (g + 1) * 512, :].rearrange(
                            "(t p) e -> p t e", p=128
                        ),
                    )

        def epilogue(b, mstat):
            r1 = statsp.tile([128, 1], F32, tag="r1", bufs=2, name=f"r1{b}")
            nc.vector.reduce_max(out=r1, in_=mstat, axis=mybir.AxisListType.X)
            r1t_ps = ps_misc.tile([1, 128], F32, tag="misc", name=f"r1tp{b}")
            nc.tensor.transpose(r1t_ps, r1, ident)
            neg_gmax = statsp.tile([1, 1], F32, tag="ngm", bufs=2, name=f"ngm{b}")
            nc.vector.reduce_max(
                out=neg_gmax, in_=r1t_ps, axis=mybir.AxisListType.X, negate=True
            )
            ngb_ps = ps_misc.tile([128, 1], F32, tag="misc", name=f"ngbp{b}")
            nc.tensor.matmul(ngb_ps, ones_r, neg_gmax, start=True, stop=True)
            ngb_sb = statsp.tile([128, 1], F32, tag="ngb", bufs=2, name=f"ngb{b}")
            nc.vector.tensor_copy(out=ngb_sb, in_=ngb_ps)
            e_sb = statsp.tile([128, NT], F32R, tag="e_sb", bufs=2, name=f"esb{b}")
            s_col = statsp.tile([128, 1], F32, tag="s_col", bufs=2, name=f"sc{b}")
            nc.scalar.activation(
                out=e_sb,
                in_=mstat,
                func=mybir.ActivationFunctionType.Exp,
                bias=ngb_sb,
                scale=1.0,
                accum_out=s_col,
            )
            tot_ps = ps_misc.tile([1, 1], F32, tag="misc", name=f"totp{b}")
            nc.tensor.matmul(tot_ps, s_col, ones_c, start=True, stop=True)
            rt_sb = statsp.tile([1, 1], F32, tag="rt", bufs=2, name=f"rt{b}")
            nc.vector.reciprocal(out=rt_sb, in_=tot_ps)
            q2c_ps = ps_q2c.tile([1, E], F32, tag="q2c", name=f"q2cp{b}")
            for t in range(NT):
                nc.tensor.matmul(
                    q2c_ps,
                    e_sb[:, t : t + 1],
                    stgs[b, t // 4][:, t % 4, 0:E],
                    start=(t == 0),
                    stop=(t == NT - 1),
                )
            q2c_sb = statsp.tile([1, E], F32, tag="q2c_sb", bufs=2, name=f"q2cs{b}")
            nc.scalar.activation(
                out=q2c_sb,
                in_=q2c_ps,
                func=mybir.ActivationFunctionType.Copy,
                scale=rt_sb,
            )
            q2cb_ps = ps_misc.tile([128, E], F32, tag="misc", name=f"q2cbp{b}")
            nc.tensor.matmul(q2cb_ps, ones_r, q2c_sb, start=True, stop=True)
            q2cb4 = statsp.tile([128, 4, E], F32, tag="q2cb4", bufs=2,
                               name=f"q2cb4{b}")
            for lane in range(4):
                nc.vector.tensor_copy(out=q2cb4[:, lane, :], in_=q2cb_ps)
            for g in range(NG):
                stg = stgs[b, g]
                nc.vector.tensor_mul(
                    stg[:, :, 3 * E : 4 * E], stg[:, :, 0:E], q2cb4
                )
                nc.sync.dma_start(
                    out=out_ext[
                        b, g * 512 : (g + 1) * 512, 3 * E : 4 * E
                    ].rearrange("(t p) f -> p t f", p=128),
                    in_=stg[:, :, 3 * E : 4 * E],
                )

        for b in range(BPC):
            mstat = statsp.tile([128, NT], F32, tag="mstat", bufs=2,
                                name=f"mstat{b}")
            row_sum = statsp.tile([128, NT], F32, tag="row_sum", bufs=2,
                                  name=f"rsum{b}")
            recip = statsp.tile([128, NT], F32, tag="recip", bufs=2,
                                name=f"recip{b}")
            qm_h, rhs_aug, qw_bcast2 = phase_a(b)
            pass1(b, qm_h, rhs_aug, qw_bcast2, mstat, row_sum, recip)
            epilogue(b, mstat)


_NC_CACHE = None


def _build():
    global _NC_CACHE
    if _NC_CACHE is not None:
        return _NC_CACHE
    nc = bacc.Bacc(
        "TRN2", target_bir_lowering=False, debug=False, num_devices=NCORES
    )
    ctx_in = nc.dram_tensor("context", [BPC, C, E], F32R, kind="ExternalInput").ap()
    q_in = nc.dram_tensor("question", [BPC, Q, E], F32, kind="ExternalInput").ap()
    wq_in = nc.dram_tensor("w_question", [E], F32, kind="ExternalInput").ap()
    wc_in = nc.dram_tensor("w_context", [E], F32, kind="ExternalInput").ap()
    wm_in = nc.dram_tensor("w_multiple", [E], F32, kind="ExternalInput").ap()
    out_ext = nc.dram_tensor("out", [BPC, C, 4 * E], F32R, kind="ExternalOutput").ap()
    with tile.TileContext(nc) as tc:
        _body(tc, out_ext, ctx_in, q_in, wq_in, wc_in, wm_in)
    nc.compile()
    _NC_CACHE = nc
    return nc


def _run(inputs, trace=False, **kw):
    nc = _build()
    context = np.ascontiguousarray(np.asarray(inputs["context"], dtype=np.float32))
    question = np.ascontiguousarray(np.asarray(inputs["question"], dtype=np.float32))
    wq = np.ascontiguousarray(np.asarray(inputs["w_question"], dtype=np.float32))
    wc = np.ascontiguousarray(np.asarray(inputs["w_context"], dtype=np.float32))
    wm = np.ascontiguousarray(np.asarray(inputs["w_multiple"], dtype=np.float32))
    in_maps = []
    for i in range(NCORES):
        sl = slice(i * BPC, (i + 1) * BPC)
        in_maps.append(
            {
                "context": context[sl],
                "question": question[sl],
                "w_question": wq,
                "w_context": wc,
                "w_multiple": wm,
            }
        )
    res = run_bass_kernel_spmd(
        nc, in_maps, core_ids=list(range(NCORES)), trace=trace, **kw
    )
    out = np.concatenate([res.results[i]["out"] for i in range(NCORES)], axis=0)
    return out, res


def kernel(**inputs):
    try:
        out, _ = _run(inputs, trace=False)
    except Exception:
        # transient device errors (e.g. a wedged core from a prior run)
        # usually clear on retry
        out, _ = _run(inputs, trace=False)
    return out
